# revision 1
# baseline (speedup 1.0000x reference)
"""Chamfer distance kernel for 8 Trainium2 NeuronCores.

Problem: xyz1 [4, 8192, 3] f32, xyz2 [4, 8192, 3] f32 ->
         (dist1 [4, 8192] f32, dist2 [4, 8192] f32)
  d[b,n,m] = max(||xyz1[b,n] - xyz2[b,m]||^2, 0)
  dist1 = min over m, dist2 = min over n.

Sharding: 2 cores per batch; each core takes 4096 xyz1 rows x all 8192
xyz2 rows. dist1 chunks are exact per core; dist2 partials are combined
with an elementwise min on the host.

Device algorithm (per core):
  The negated distance e[n,m] = 2*x.x' - ||x||^2 - ||x'||^2 is produced
  by a single K=18 matmul over host-prepared feature matrices.  Features
  are bf16 hi/lo splits (coords 2-way x 4 product terms, squared norms
  3-way against an exact 1.0) so the bf16 TensorEngine matmul reproduces
  fp32-accurate distances (abs err ~1e-6) at full PE rate.
    - PE: e tiles [128 n, 512 m] into PSUM fp32, grouped 4 tiles/bank-quad.
    - ACT: cast-copy PSUM fp32 -> SBUF fp16 (enables DVE 2x mode).
    - DVE: running max over m-groups (dist1, + one tensor_reduce per
      n-tile) and running max over n-tiles per m column (dist2).
    - PE transpose + DVE reduce turn the dist2 accumulator [128, 8192]
      into per-m minima; Relu(-x) applies the negation + zero clamp.
"""

import numpy as np
import ml_dtypes
from contextlib import ExitStack

import concourse.bass as bass
import concourse.bacc as bacc
import concourse.tile as tile
from concourse import mybir
from concourse.bass_utils import run_bass_kernel_spmd
from concourse.masks import make_identity

B = 4
N = 8192
M = 8192
NCORES = 8
NH = N // 2          # xyz1 rows per core
K = 18               # feature rows
NT = NH // 128       # 32 n-tiles
GW = 2048            # m-group width (4 PSUM banks)
MG = M // GW         # 4 m-groups
MMW = 512            # matmul moving width

F32 = mybir.dt.float32
F16 = mybir.dt.float16
BF16 = mybir.dt.bfloat16

_CACHE = {}


def _build_nc():
    nc = bacc.Bacc(
        "TRN2", target_bir_lowering=False, debug=False, enable_asserts=False
    )
    a_d = nc.dram_tensor("a_feat", [K, NH], BF16, kind="ExternalInput").ap()
    b_d = nc.dram_tensor("b_feat", [K, M], BF16, kind="ExternalInput").ap()
    d1_d = nc.dram_tensor("dist1", [NH], F32, kind="ExternalOutput").ap()
    d2_d = nc.dram_tensor("dist2", [M], F32, kind="ExternalOutput").ap()

    with tile.TileContext(nc) as tc, ExitStack() as ctx:
        const_pool = ctx.enter_context(tc.tile_pool(name="const", bufs=1))
        feat_pool = ctx.enter_context(tc.tile_pool(name="feat", bufs=1))
        acc_pool = ctx.enter_context(tc.tile_pool(name="acc", bufs=1))
        a1_pool = ctx.enter_context(tc.tile_pool(name="a1", bufs=2))
        e16_pool = ctx.enter_context(tc.tile_pool(name="e16", bufs=3))
        outp_pool = ctx.enter_context(tc.tile_pool(name="outp", bufs=1))
        psum_pool = ctx.enter_context(tc.tile_pool(name="ps", bufs=2, space="PSUM"))

        a_sb = feat_pool.tile([K, NH], BF16, tag="a_sb")
        b_sb = feat_pool.tile([K, M], BF16, tag="b_sb")
        nc.sync.dma_start(a_sb[:], a_d[:, :])
        nc.sync.dma_start(b_sb[:], b_d[:, :])

        id16 = const_pool.tile([128, 128], F16, tag="id16")
        make_identity(nc, id16[:])
        id32 = const_pool.tile([128, 128], F32, tag="id32")
        make_identity(nc, id32[:])

        acc2 = acc_pool.tile([128, M], F16, tag="acc2")
        d1col = acc_pool.tile([128, NT], F32, tag="d1col")
        d2col = acc_pool.tile([128, M // 128], F32, tag="d2col")

        for i in range(NT):
            acc1 = a1_pool.tile([128, GW], F16, tag="acc1")
            for g in range(MG):
                eg = psum_pool.tile([128, GW], F32, tag="eg")
                for jj in range(GW // MMW):
                    mj = g * GW + jj * MMW
                    nc.tensor.matmul(
                        eg[:, jj * MMW : (jj + 1) * MMW],
                        a_sb[:, i * 128 : (i + 1) * 128],
                        b_sb[:, mj : mj + MMW],
                        start=True,
                        stop=True,
                    )
                e16 = e16_pool.tile([128, GW], F16, tag="e16")
                nc.scalar.copy(e16[:], eg[:])
                # dist1 running max over m
                if g == 0:
                    nc.vector.tensor_copy(acc1[:], e16[:])
                else:
                    nc.vector.tensor_tensor(
                        acc1[:], acc1[:], e16[:], mybir.AluOpType.max
                    )
                # dist2 running max over n
                sl = acc2[:, g * GW : (g + 1) * GW]
                if i == 0:
                    nc.vector.tensor_copy(sl, e16[:])
                else:
                    nc.vector.tensor_tensor(sl, sl, e16[:], mybir.AluOpType.max)
            nc.vector.tensor_reduce(
                d1col[:, i : i + 1],
                acc1[:],
                axis=mybir.AxisListType.X,
                op=mybir.AluOpType.max,
            )

        # dist2: cross-partition min via PE transpose + free-dim reduce
        for blk in range(M // 128):
            tp = psum_pool.tile([128, 128], F16, tag="eg")
            nc.tensor.transpose(tp[:], acc2[:, blk * 128 : (blk + 1) * 128], id16[:])
            nc.vector.tensor_reduce(
                d2col[:, blk : blk + 1],
                tp[:],
                axis=mybir.AxisListType.X,
                op=mybir.AluOpType.max,
            )

        # outputs: transpose so DMA rows are contiguous, then relu(-x)
        tp1 = psum_pool.tile([NT, 128], F32, tag="eg")
        nc.tensor.transpose(tp1[:], d1col[:], id32[:])
        o1 = outp_pool.tile([NT, 128], F32, tag="o1")
        nc.scalar.activation(
            o1[:], tp1[:], mybir.ActivationFunctionType.Relu, bias=0.0, scale=-1.0
        )
        nc.sync.dma_start(d1_d.rearrange("(i p) -> i p", p=128), o1[:])

        tp2 = psum_pool.tile([M // 128, 128], F32, tag="eg")
        nc.tensor.transpose(tp2[:], d2col[:], id32[:])
        o2 = outp_pool.tile([M // 128, 128], F32, tag="o2")
        nc.scalar.activation(
            o2[:], tp2[:], mybir.ActivationFunctionType.Relu, bias=0.0, scale=-1.0
        )
        nc.sync.dma_start(d2_d.rearrange("(i p) -> i p", p=128), o2[:])

    nc.compile()
    return nc


def _split2(x):
    """fp32 -> (hi, lo) bf16 pair with hi+lo ~ x (rel err 2^-18)."""
    hi = x.astype(ml_dtypes.bfloat16)
    lo = (x - hi.astype(np.float32)).astype(ml_dtypes.bfloat16)
    return hi, lo


def _split3(x64):
    """f64 -> three bf16 values summing to x to ~2^-27 rel."""
    a = x64.astype(ml_dtypes.bfloat16)
    r = x64 - a.astype(np.float64)
    b = r.astype(ml_dtypes.bfloat16)
    r = r - b.astype(np.float64)
    c = r.astype(ml_dtypes.bfloat16)
    return a, b, c


def _features(x1, x2):
    """x1 [NH,3] f32, x2 [M,3] f32 -> (A [K,NH] bf16, B [K,M] bf16).

    sum_k A[k,n]*B[k,m] = 2*x1[n].x2[m] - ||x1[n]||^2 - ||x2[m]||^2
    """
    nh, m = x1.shape[0], x2.shape[0]
    uh, ul = _split2(2.0 * x1)           # [nh,3] each
    vh, vl = _split2(x2)                 # [m,3]
    s0, s1, s2 = _split3(-np.sum(x1.astype(np.float64) ** 2, axis=1))
    t0, t1, t2 = _split3(-np.sum(x2.astype(np.float64) ** 2, axis=1))

    one_n = np.ones(nh, ml_dtypes.bfloat16)
    one_m = np.ones(m, ml_dtypes.bfloat16)

    A = np.empty((K, nh), ml_dtypes.bfloat16)
    Bm = np.empty((K, m), ml_dtypes.bfloat16)
    A[0:3] = uh.T
    Bm[0:3] = vh.T
    A[3:6] = uh.T
    Bm[3:6] = vl.T
    A[6:9] = ul.T
    Bm[6:9] = vh.T
    A[9:12] = ul.T
    Bm[9:12] = vl.T
    A[12], A[13], A[14] = s0, s1, s2
    Bm[12] = Bm[13] = Bm[14] = one_m
    A[15] = A[16] = A[17] = one_n
    Bm[15], Bm[16], Bm[17] = t0, t1, t2
    return np.ascontiguousarray(A), np.ascontiguousarray(Bm)


def kernel(xyz1, xyz2):
    xyz1 = np.asarray(xyz1, dtype=np.float32)
    xyz2 = np.asarray(xyz2, dtype=np.float32)
    assert xyz1.shape == (B, N, 3) and xyz2.shape == (B, M, 3)

    if "nc" not in _CACHE:
        _CACHE["nc"] = _build_nc()
    nc = _CACHE["nc"]

    in_maps = []
    for core in range(NCORES):
        b, half = divmod(core, 2)
        A, Bm = _features(xyz1[b, half * NH : (half + 1) * NH], xyz2[b])
        in_maps.append({"a_feat": A, "b_feat": Bm})

    res = run_bass_kernel_spmd(nc, in_maps, core_ids=list(range(NCORES))).results

    dist1 = np.empty((B, N), np.float32)
    dist2 = np.empty((B, M), np.float32)
    for b in range(B):
        dist1[b, :NH] = res[2 * b]["dist1"]
        dist1[b, NH:] = res[2 * b + 1]["dist1"]
        dist2[b] = np.minimum(res[2 * b]["dist2"], res[2 * b + 1]["dist2"])
    return dist1, dist2
